# revision 10
# baseline (speedup 1.0000x reference)
"""Trainium2 Bass kernel: 7x7 valid cross-correlation (Conv2D) + bias on a
4096x4096 fp32 image, sharded over 8 NeuronCores as a 2x4 (rows x cols) grid.

Per core: output block of 2045 x 1023 (padded to 1024 cols), input block of
2051 x 1029 (halo included, padded to 1040 cols), all device-side tensors in
bf16 (accumulation in fp32 PSUM).

Algorithm per core:
  - Output rows are processed in tiles of 122 (=128-6) rows; 17 tiles.
  - For each row-tile and each 512-wide output column chunk (2 chunks), the
    2D conv is 7 accumulating TensorE matmuls (one per horizontal tap b):
        psum[m, n] += B_b.T @ x[:, n+b]
    where B_b[k, m] = w[k-m, b] is a banded [128 x 122] bf16 matrix that
    performs the 7-tap vertical convolution for kernel column b.
  - PSUM (fp32) is evacuated by the VectorE with a fused bias add and a
    cast to bf16 (tensor_scalar_add).
  - All DRAM reads/writes are fully contiguous slabs so each dma_start is
    split across all 16 SDMA engines; stores go through SWDGE (gpsimd)
    because HWDGE pins SBUF->HBM writes onto 2 engines (~50 GB/s).
  - 8 dummy matmuls on a zeroed tile run during the ~7us engine-init
    preamble to warm the PE HAM clock gate (1.2 -> 2.4 GHz) before the
    first real matmul.
Weight bands/bias are built host-side and replicated to all cores.
"""

import sys

sys.path.insert(0, "/opt/trn_rl_repo")

import numpy as np
import ml_dtypes

import concourse.bass as bass
import concourse.bacc as bacc
import concourse.mybir as mybir
from concourse.tile import TileContext
from concourse.bass_utils import run_bass_kernel_spmd

BF16 = ml_dtypes.bfloat16

KH, KW = 7, 7
H, W = 4096, 4096
OH, OW = H - KH + 1, W - KW + 1  # 4090, 4090

RB, CB = 2, 4                    # core grid: 2 row blocks x 4 col blocks
CORE_OR = 2045                   # output rows per core (2*2045 = 4090)
CORE_OC = 1023                   # logical output cols per core (4*1023 >= 4090)
OC_PAD = 1024                    # padded output cols (contiguous 2KB rows)
CORE_IR = CORE_OR + KH - 1       # 2051 input rows
CORE_IC = CORE_OC + KW - 1       # 1029 input cols
IC_PAD = 1040                    # padded input cols (32B-aligned 2080B rows)
TILE_R = 128 - (KH - 1)          # 122 output rows per row-tile
CHUNK = 512                      # output cols per PSUM bank (fp32)
N_WARM = 8                       # HAM warm-up matmuls (~3.4us at cold clock)

_NC_CACHE = {}


def _build_nc():
    """Build the single-core Bass program (SPMD: same program on all cores)."""
    f32 = mybir.dt.float32
    bf16 = mybir.dt.bfloat16
    kin = TILE_R + KH - 1  # 128 input rows per full tile
    assert kin == 128
    n_tiles = -(-CORE_OR // TILE_R)   # 17
    n_chunks = OC_PAD // CHUNK        # 2

    nc = bacc.Bacc()
    x_in = nc.declare_dram_parameter("x_in", [CORE_IR, IC_PAD], bf16, isOutput=False)
    bands = nc.declare_dram_parameter("bands", [kin, KW * TILE_R], bf16, isOutput=False)
    biasb = nc.declare_dram_parameter("biasb", [128, 1], f32, isOutput=False)
    # padded to a whole number of 122-row tiles: SWDGE stores with fewer than
    # 122 partitions collapse onto a single SDMA engine (~26 GB/s), so every
    # store is a uniform [122, 2048B] slab; host ignores rows >= CORE_OR.
    or_pad = n_tiles * TILE_R  # 2074
    y_out = nc.declare_dram_parameter("y_out", [or_pad, OC_PAD], bf16, isOutput=True)

    with TileContext(nc) as tc:
        with (
            tc.tile_pool(name="const", bufs=1) as cpool,
            tc.tile_pool(name="xio", bufs=6) as xpool,
            tc.tile_pool(name="yio", bufs=6) as ypool,
            tc.tile_pool(name="ps", bufs=8, space="PSUM") as ppool,
        ):
            band_sb = cpool.tile([kin, KW * TILE_R], bf16)
            bias_sb = cpool.tile([128, 1], f32)
            dummy = cpool.tile([128, CHUNK], bf16)

            # HAM warm-up: no data deps, so these run during the engine-init
            # preamble and the first input DMA, un-throttling the PE clock.
            nc.vector.memset(dummy[:, :], 0.0)
            ps_w = ppool.tile([128, CHUNK], f32, tag="ps")
            for i in range(N_WARM):
                nc.tensor.matmul(
                    ps_w[:, :],
                    lhsT=dummy[:, :128],
                    rhs=dummy[:, :],
                    start=(i == 0),
                    stop=(i == N_WARM - 1),
                )

            nc.sync.dma_start(out=band_sb[:, :], in_=bands[:, :])
            nc.sync.dma_start(out=bias_sb[:, :], in_=biasb[:, :])

            for t in range(n_tiles):
                r0 = t * TILE_R
                h = min(TILE_R, CORE_OR - r0)
                kh = h + KH - 1
                x_sb = xpool.tile([kin, IC_PAD], bf16, tag="x")
                if t == 0:
                    # split the first load by columns: chunk-0 MMs only need
                    # cols < 520, so they start ~0.6us earlier
                    nc.sync.dma_start(
                        out=x_sb[:kh, :520], in_=x_in[r0 : r0 + kh, :520]
                    )
                    nc.sync.dma_start(
                        out=x_sb[:kh, 520:], in_=x_in[r0 : r0 + kh, 520:]
                    )
                else:
                    nc.sync.dma_start(out=x_sb[:kh, :], in_=x_in[r0 : r0 + kh, :])
                y_sb = ypool.tile([128, OC_PAD], bf16, tag="y")
                last = t == n_tiles - 1
                for j in range(n_chunks):
                    c0 = j * CHUNK
                    ps = ppool.tile([128, CHUNK], f32, tag="ps")
                    for b in range(KW):
                        nc.tensor.matmul(
                            ps[:h, :],
                            lhsT=band_sb[:kh, b * TILE_R : b * TILE_R + h],
                            rhs=x_sb[:kh, c0 + b : c0 + b + CHUNK],
                            start=(b == 0),
                            stop=(b == KW - 1),
                        )
                    nc.vector.tensor_scalar_add(
                        y_sb[:h, c0 : c0 + CHUNK], ps[:h, :], bias_sb[:h, 0:1]
                    )
                    if last:
                        # final tile: store each column chunk as soon as it is
                        # evacuated so only half the store trails the last MM;
                        # row-halves go to the two HWDGE rings (SP + ACT) so
                        # their engine pairs run in parallel
                        h1 = h // 2
                        nc.sync.dma_start(
                            out=y_out[r0 : r0 + h1, c0 : c0 + CHUNK],
                            in_=y_sb[:h1, c0 : c0 + CHUNK],
                        )
                        nc.scalar.dma_start(
                            out=y_out[r0 + h1 : r0 + h, c0 : c0 + CHUNK],
                            in_=y_sb[h1:h, c0 : c0 + CHUNK],
                        )
                if not last:
                    # SWDGE (gpsimd) for stores: HWDGE pins SBUF->HBM writes
                    # onto 2 of the 16 SDMA engines; SWDGE rotates engine
                    # pairs per store, giving enough aggregate bandwidth when
                    # pipelined. Store the full 122 rows so the descriptor
                    # shape is uniform.
                    nc.gpsimd.dma_start(
                        out=y_out[r0 : r0 + TILE_R, :], in_=y_sb[:TILE_R, :]
                    )
    nc.compile()
    return nc


def _make_bands(weight):
    """B_b[k, m] = w[k-m, b] laid out as [128, KW*TILE_R] (band b in cols
    [b*TILE_R, (b+1)*TILE_R))."""
    kin = TILE_R + KH - 1
    bands = np.zeros((kin, KW * TILE_R), np.float32)
    m = np.arange(TILE_R)
    for b in range(KW):
        for a in range(KH):
            bands[m + a, b * TILE_R + m] = weight[a, b]
    return bands.astype(BF16)


def _shard_inputs(x, weight, bias):
    bands = _make_bands(weight)
    biasb = np.full((128, 1), np.float32(bias[0]), np.float32)
    xb = x.astype(BF16)
    in_maps = []
    for rb in range(RB):
        for cb in range(CB):
            r0, c0 = rb * CORE_OR, cb * CORE_OC
            rr = min(CORE_IR, H - r0)
            cc = min(CORE_IC, W - c0)
            xt = np.zeros((CORE_IR, IC_PAD), BF16)
            xt[:rr, :cc] = xb[r0 : r0 + rr, c0 : c0 + cc]
            in_maps.append({"x_in": xt, "bands": bands, "biasb": biasb})
    return in_maps


def _assemble(results):
    out = np.empty((OH, OW), np.float32)
    i = 0
    for rb in range(RB):
        for cb in range(CB):
            r0, c0 = rb * CORE_OR, cb * CORE_OC
            rr = min(CORE_OR, OH - r0)
            cc = min(CORE_OC, OW - c0)
            out[r0 : r0 + rr, c0 : c0 + cc] = results[i]["y_out"][:rr, :cc].astype(
                np.float32
            )
            i += 1
    return out


def _get_nc():
    if "nc" not in _NC_CACHE:
        _NC_CACHE["nc"] = _build_nc()
    return _NC_CACHE["nc"]


def _run(x, weight, bias, **spmd_kwargs):
    x = np.ascontiguousarray(np.asarray(x), dtype=np.float32)
    weight = np.asarray(weight, dtype=np.float32)
    bias = np.asarray(bias, dtype=np.float32)
    in_maps = _shard_inputs(x, weight, bias)
    res = run_bass_kernel_spmd(_get_nc(), in_maps, list(range(RB * CB)), **spmd_kwargs)
    return _assemble(res.results), res


def kernel(x, weight, bias):
    out, _ = _run(x, weight, bias)
    return out
